# revision 42
# baseline (speedup 1.0000x reference)
"""Trainium2 Bass kernel for DetectPeaks (sliding-window NMS + top-2).

Computes, for xcorr [32, 3, 64, 8192] f32:
    x = |xcorr|
    smax = sliding max over time, window 301 (centered, clipped)
    scores = where(smax == x, x, 0)
    top2 values + indices along time  -> ([32,3,64,2] f32, [32,3,64,2] int32)

Strategy: flatten to 6144 independent rows, shard 768 rows per core across
8 cores (data parallel, no communication).  Per 128-row tile:
  - abs on the scalar engine (in place, in a 0.0-padded buffer)
  - van Herk / Gil-Werman sliding max at half resolution: per-150-block
    prefix/suffix max scans on h[v] = max(x[2v], x[2v+1])
    (tensor_tensor_scan with op=max on DVE)
  - masked scores at PAIR resolution: smax is split per parity into two
    contiguous half-res arrays, each masked with a fused custom-DVE
    select, then combined with one pairwise max ->
    mp[v] = (pair v holds a peak) ? peak value : 0      [128, 4096]
  - top-2 extraction: peaks are >= 151 apart, so every 64-sample block
    (32 pair elements) holds at most one nonzero of mp.  Block "sums"
    (== the peak value, bit exactly) are computed on the otherwise-idle
    TENSOR engine: per 128-wide chunk, PE-transpose -> PSUM, ScalarE
    copy -> SBUF, then a selector matmul accumulates the chunk's four
    64-blocks into a PSUM S^T[128 blocks, 128 rows] tile.  After a
    final transpose, max8 over the 128 block maxes gives the top-8
    peak values, and one find_index8 over the |x| row (the xp tile,
    already resident) gives exact indices (minus the pad offset).
    The max8/find_index8 tail of tile i is emitted after tile i+1's
    scan chain so the in-order DVE stream never waits on the PE chain.

Exact-duplicate f32 values in the input (birthday collisions) need care:
  - two tied peaks in one 64-block make the PE sum a doubled "ghost"
    whose find_index8 lookup misses; run() drops ghosts (idx >= NT).
  - find_index8 duplicate search values return successive occurrences,
    matching jax.lax.top_k's ascending-index order for tied peaks.
"""

import numpy as np

NB, NC, NX, NT = 32, 3, 64, 8192
KERNEL = 301
PAD = KERNEL // 2  # 150
N_CORES = 8
ROWS = NB * NC * NX  # 6144
ROWS_PER_CORE = ROWS // N_CORES  # 768
P_DIM = 128
NTILE = ROWS_PER_CORE // P_DIM  # 6
BMC = 64   # time-domain block size for the block-max top-k (64 < 151)
NBM = NT // BMC  # 128 block maxes per row
BLK = BMC // 2   # 32 pair elements per block
PER = P_DIM // BLK  # 4 blocks per transpose chunk
WOFF = P_DIM - PER

_cached = None
_select_ge = None


def _register_select_ge():
    """Register a fused custom-DVE op  out = (in0 >= in1) ? in0 : 0  at
    runtime (the package's dve_ops.py is read-only; the documented way to
    add an op is appending to its OPS registry)."""
    global _select_ge
    if _select_ge is not None:
        return _select_ge
    import concourse.dve_ops as dve_ops_mod
    from concourse.dve_spec import Spec, Src0, Src1, Zero, select, lower
    from concourse.dve_spec import _has_src1
    from concourse.dve_uop import DveOpSpec

    name = "SELECT_GE_PEAK_ANT"
    for op in dve_ops_mod.OPS:
        if op.name == name:
            _select_ge = op
            return op

    spec = Spec(
        body=select(Src0 >= Src1, Src0, Zero),
        reference=lambda in0, in1, s0, s1, imm2: np.where(
            in0 >= in1, in0, 0.0
        ).astype(np.float32),
    )
    row = dve_ops_mod._CUSTOM_DVE_ROW_BASE + len(dve_ops_mod.OPS)
    assert row < 0x20
    shas = {}
    for ver in ("v3", "v4"):
        s = DveOpSpec(
            name=name, opcode=row, uops=lower(spec, ver=ver), rd1_en=_has_src1(spec)
        )
        shas[ver] = s.sha(ver)
    op = dve_ops_mod.DveOp(name, spec, subdim=False, uops_sha=shas)
    dve_ops_mod.OPS.append(op)
    dve_ops_mod._SUB_OPCODE_FOR_NAME[name] = row
    dve_ops_mod.CUSTOM_DVE_SPECS[name] = spec
    _select_ge = op
    return op


def _build(rows_per_core=ROWS_PER_CORE):
    import concourse.mybir as mybir
    from concourse.bacc import Bacc
    from concourse.tile import TileContext
    from concourse.masks import make_identity

    f32 = mybir.dt.float32
    Alu = mybir.AluOpType
    Act = mybir.ActivationFunctionType
    n_tiles = rows_per_core // P_DIM
    sel_op = _register_select_ge()

    # Bacc (not plain Bass): its finalize() runs generate_event_semaphores,
    # which splits multi-sem waits into EventSemaphore prefixes — TRN2
    # instructions only have a single wait slot.
    nc = Bacc(None, target_bir_lowering=False)
    x_in = nc.dram_tensor("x", [rows_per_core, NT], f32, kind="ExternalInput")
    w_in = nc.dram_tensor("wb", [P_DIM, 2 * P_DIM], f32, kind="ExternalInput")
    out_vals = nc.dram_tensor("out_vals", [rows_per_core, 8], f32, kind="ExternalOutput")
    out_idx = nc.dram_tensor("out_idx", [rows_per_core, 8], mybir.dt.uint32, kind="ExternalOutput")

    # Half-resolution (parity) van Herk: the expensive segmented scans run
    # on h[v] = max(x[2v], x[2v+1]) with window 150 / block 150, then the
    # per-parity sliding max is
    #   smax[2u]   = max(H150[u], xp[2u+300])
    #   smax[2u+1] = max(xp[2u+1], H150[u+1])
    # with H150[v] = max(h[v..v+149]) = max(Sh[v], Ph[v+149]).
    B2 = 150
    MH = NT // 2 + 1        # 4097 H150 values needed
    PHE = B2 - 1 + MH       # 4246, Ph read range
    SHE = (NT // 2 // B2) * B2 + B2 - 1  # 4199, end of Sh's block
    HLEN = PHE + 1          # 4247 h values suffice for both scans
    LP2 = 2 * HLEN          # 8494 xp values (>= PAD + NT + PAD = 8492)
    GLEN = PHE + 1
    HNT = NT // 2

    with TileContext(nc) as tc:
        with (
            tc.tile_pool(name="const", bufs=1) as cpool,
            tc.tile_pool(name="big", bufs=3) as bigpool,
            tc.tile_pool(name="scan", bufs=1) as scanpool,
            tc.tile_pool(name="pair", bufs=2) as pairpool,
            tc.tile_pool(name="small", bufs=2) as smallpool,
            tc.tile_pool(name="stage", bufs=2) as stagepool,
            tc.tile_pool(name="ps", bufs=3, space="PSUM") as pspool,
            tc.tile_pool(name="psS", bufs=1, space="PSUM") as psSpool,
            tc.tile_pool(name="ps2", bufs=2, space="PSUM") as ps2pool,
        ):
            # Segment mask for block-restarting max scans over h: zeros at
            # multiples of 150 (scan state = max(G2[v]*state, h[v]) restarts
            # at every 0 since all data >= 0). G2[1:] reversed provides the
            # restart markers for the reversed (suffix) scan.
            G2 = cpool.tile([P_DIM, GLEN], f32, tag="G2")
            nc.gpsimd.memset(G2[:, :], 1.0)
            nc.gpsimd.memset(G2[:, 0:GLEN:B2], 0.0)
            ident = cpool.tile([P_DIM, P_DIM], f32, tag="ident")
            make_identity(nc, ident[:, :])
            # Wbig[p, k] = 1 iff k == WOFF + p//BLK; the slice
            # Wbig[:, WOFF-PER*c : WOFF-PER*c+128] is the chunk-c selector
            # placing its PER blocks at rows PER*c.. of the accumulated S^T.
            Wbig = cpool.tile([P_DIM, 2 * P_DIM], f32, tag="W")
            nc.sync.dma_start(Wbig[:, :], w_in[:, :])

            prev = None  # deferred tail state: (rows, xint, Ssb)
            for i in range(n_tiles):
                rows = slice(i * P_DIM, (i + 1) * P_DIM)
                xp = bigpool.tile([P_DIM, LP2], f32, tag="xp")
                xint = xp[:, PAD:PAD + NT]
                # Pads + abs all on the scalar engine (|0|=0 keeps pads valid);
                # pads only matter as neutral (<= data) elements.  Tile 0 is
                # fully on the critical path (nothing to overlap with), so
                # chunk its DMA+abs to let compute start sooner.
                h = scanpool.tile([P_DIM, HLEN], f32, tag="h")
                Ph = scanpool.tile([P_DIM, PHE], f32, tag="Ph")
                # Chunked DMA+abs on tile 0 (nothing hides its DMA) so DVE
                # h-build / prefix-scan can start after the first chunk.
                edges = [0, 2048, 4096, 6144, NT] if i == 0 else [0, NT]
                nchunk = len(edges) - 1
                nc.scalar.memzero(xp[:, 0:PAD])
                nc.scalar.memzero(xp[:, PAD + NT:LP2])
                # h-build boundaries per DMA chunk (pair coords); the first
                # chunk also covers the left pad, the last the right pad.
                hb = [0] + [(PAD + edges[c + 1]) // 2 for c in range(nchunk - 1)] + [HLEN]
                pb = [min(b, PHE) for b in hb]
                for c in range(nchunk):
                    sl = slice(PAD + edges[c], PAD + edges[c + 1])
                    nc.sync.dma_start(xp[:, sl], x_in[rows, edges[c]:edges[c + 1]])
                    nc.scalar.activation(xp[:, sl], xp[:, sl], Act.Abs)
                    # h over the freshly-landed pairs, then the prefix scan
                    # over the same range (state chained via `initial`).
                    nc.vector.tensor_tensor(
                        out=h[:, hb[c]:hb[c + 1]],
                        in0=xp[:, 2 * hb[c]:2 * hb[c + 1]:2],
                        in1=xp[:, 2 * hb[c] + 1:2 * hb[c + 1]:2],
                        op=Alu.max,
                    )
                    if pb[c + 1] > pb[c]:
                        nc.vector.tensor_tensor_scan(
                            Ph[:, pb[c]:pb[c + 1]], G2[:, pb[c]:pb[c + 1]],
                            h[:, pb[c]:pb[c + 1]],
                            0.0 if c == 0 else Ph[:, pb[c] - 1:pb[c]],
                            op0=Alu.mult, op1=Alu.max,
                        )

                # Trimmed scan ranges: Ph is only read on [149, 4246) and Sh
                # on [0, 4097).  The suffix scan runs IN PLACE over h (h is
                # dead after the scans; streaming reads each element before
                # its overwrite).
                nc.vector.tensor_tensor_scan(
                    h[:, SHE::-1], G2[:, 1:SHE + 2][:, ::-1], h[:, SHE::-1], 0.0,
                    op0=Alu.mult, op1=Alu.max,
                )

                # H150[v] = max(Sh[v], Ph[v+149]), v in [0, 4097)
                mh = scanpool.tile([P_DIM, MH], f32, tag="mh")
                nc.vector.tensor_tensor(
                    out=mh[:, :], in0=h[:, 0:MH], in1=Ph[:, B2 - 1:B2 - 1 + MH],
                    op=Alu.max,
                )
                # Per-parity sliding max, contiguous half-res outputs.
                # smaxE reuses h's storage, smaxO reuses Ph's (both dead,
                # consumed immediately in order by the selects below).
                nc.vector.tensor_tensor(
                    out=h[:, 0:HNT], in0=mh[:, 0:HNT],
                    in1=xp[:, 2 * PAD:2 * PAD + NT:2], op=Alu.max,
                )
                nc.vector.tensor_tensor(
                    out=Ph[:, 0:HNT], in0=xp[:, 1:NT:2], in1=mh[:, 1:HNT + 1],
                    op=Alu.max,
                )
                # Masked scores per parity (fused select), then one pairwise
                # max -> mp.  selE reuses mh's storage, selO overwrites Ph.
                nc.vector._custom_dve(
                    sel_op, out=mh[:, 0:HNT],
                    in0=xp[:, PAD:PAD + NT:2], in1=h[:, 0:HNT],
                )
                nc.vector._custom_dve(
                    sel_op, out=Ph[:, 0:HNT],
                    in0=xp[:, PAD + 1:PAD + NT:2], in1=Ph[:, 0:HNT],
                )
                mp = pairpool.tile([P_DIM, HNT], f32, tag="mp")
                nc.vector.tensor_tensor(
                    out=mp[:, :], in0=mh[:, 0:HNT], in1=Ph[:, 0:HNT],
                    op=Alu.max,
                )

                if i == n_tiles - 1:
                    # Last tile: nothing left to hide the PE chain behind, so
                    # take the top-8 directly from mp on DVE (4096-elem max8
                    # beats waiting ~20us for the PE pipeline to drain).
                    if prev is not None:
                        _emit_tail(nc, out_vals, out_idx, smallpool, mybir, *prev)
                    v8 = smallpool.tile([P_DIM, 8], f32, tag="v8")
                    i8 = smallpool.tile([P_DIM, 8], mybir.dt.uint32, tag="i8")
                    nc.vector.max(out=v8, in_=mp[:, :])
                    nc.vector.max_index(out=i8, in_max=v8, in_values=xint)
                    nc.sync.dma_start(out_vals[rows, :], v8)
                    nc.sync.dma_start(out_idx[rows, :], i8)
                    prev = None
                    continue

                # ---- PE block-max pipeline (no DVE involvement) ----
                ST_ps = ps2pool.tile([P_DIM, P_DIM], f32, tag="ST")
                nch = HNT // P_DIM  # 32
                for c in range(nch):
                    tp = pspool.tile([P_DIM, P_DIM], f32, tag="tp")
                    nc.tensor.transpose(
                        tp[:, :], mp[:, c * P_DIM:(c + 1) * P_DIM], ident[:, :]
                    )
                    ts = stagepool.tile([P_DIM, P_DIM], f32, tag="ts")
                    nc.scalar.activation(ts[:, :], tp[:, :], Act.Copy)
                    nc.tensor.matmul(
                        ST_ps[:, :],
                        Wbig[:, WOFF - PER * c:WOFF - PER * c + P_DIM],
                        ts[:, :],
                        start=(c == 0), stop=(c == nch - 1),
                    )
                STs = stagepool.tile([P_DIM, P_DIM], f32, tag="STs")
                nc.scalar.activation(STs[:, :], ST_ps[:, :], Act.Copy)
                S_ps = psSpool.tile([P_DIM, P_DIM], f32, tag="S")
                nc.tensor.transpose(S_ps[:, :], STs[:, :], ident[:, :])
                Ssb = smallpool.tile([P_DIM, NBM], f32, tag="Ssb")
                nc.scalar.activation(Ssb[:, :], S_ps[:, :], Act.Copy)

                # ---- deferred DVE tail of the PREVIOUS tile ----
                if prev is not None:
                    _emit_tail(nc, out_vals, out_idx, smallpool, mybir, *prev)
                prev = (rows, xint, Ssb)
            if prev is not None:
                _emit_tail(nc, out_vals, out_idx, smallpool, mybir, *prev)
    return nc


def _emit_tail(nc, out_vals, out_idx, smallpool, mybir, rows, xint, Ssb):
    """Top-8 of the 128 block maxes; exact indices by searching the |x| row."""
    v8 = smallpool.tile([P_DIM, 8], mybir.dt.float32, tag="v8")
    i8 = smallpool.tile([P_DIM, 8], mybir.dt.uint32, tag="i8")
    nc.vector.max(out=v8, in_=Ssb[:, :])
    nc.vector.max_index(out=i8, in_max=v8, in_values=xint)
    nc.sync.dma_start(out_vals[rows, :], v8)
    nc.sync.dma_start(out_idx[rows, :], i8)


def _get_module():
    global _cached
    if _cached is None:
        _cached = _build()
        # run_bass_via_pjrt serializes the module as-is; Bacc.finalize()
        # runs register allocation + event-semaphore legalization.
        _cached.finalize()
    return _cached


def run(xcorr: np.ndarray, trace: bool = False, **spmd_kwargs):
    from concourse.bass_utils import run_bass_kernel_spmd

    x = np.ascontiguousarray(np.asarray(xcorr, dtype=np.float32).reshape(ROWS, NT))
    nc = _get_module()
    # Wbig[p, k] = 1 iff k == WOFF + p//BLK (block selector, see _build)
    wb = np.zeros((P_DIM, 2 * P_DIM), np.float32)
    p = np.arange(P_DIM)
    wb[p, WOFF + p // BLK] = 1.0
    in_maps = [
        {"x": x[c * ROWS_PER_CORE:(c + 1) * ROWS_PER_CORE], "wb": wb}
        for c in range(N_CORES)
    ]
    for attempt in range(2):
        res = run_bass_kernel_spmd(
            nc, in_maps, core_ids=list(range(N_CORES)), trace=trace, **spmd_kwargs
        )
        vals8 = np.concatenate([r["out_vals"] for r in res.results], axis=0)
        idx8 = np.concatenate([r["out_idx"] for r in res.results], axis=0)
        # find_index8 indices are relative to the searched AP view (the row
        # interior), so no pad adjustment is needed.  Ghosts (doubled tie
        # values from the PE block sum) miss in find_index8 and return the
        # 0xFFFFFFFF sentinel; drop them, keeping the first two valid
        # candidates per row.
        ghost = idx8 >= np.uint32(NT)
        if ghost.any():
            order = np.argsort(ghost, axis=1, kind="stable")[:, :2]
            vals = np.take_along_axis(vals8, order, 1)
            idx = np.take_along_axis(idx8, order, 1)
        else:
            vals = vals8[:, :2]
            idx = idx8[:, :2]
        # Reference-free sanity check (guards against rare transient device
        # glitches): final indices in range, values strictly positive and
        # non-increasing.  Re-execute once on violation.
        ok = (
            (idx < np.uint32(NT)).all()
            and (vals > 0).all()
            and (vals[:, 0] >= vals[:, 1]).all()
            and np.isfinite(vals).all()
        )
        if ok or attempt == 1:
            break
    topk_score = vals.reshape(NB, NC, NX, 2).astype(np.float32)
    topk_idx = idx.reshape(NB, NC, NX, 2).astype(np.int32)
    return (topk_score, topk_idx), res


def kernel(xcorr: np.ndarray, nlag=None, **_unused):
    out, _ = run(xcorr)
    return out


# revision 45
# speedup vs baseline: 1.0036x; 1.0036x over previous
"""Trainium2 Bass kernel for DetectPeaks (sliding-window NMS + top-2).

Computes, for xcorr [32, 3, 64, 8192] f32:
    x = |xcorr|
    smax = sliding max over time, window 301 (centered, clipped)
    scores = where(smax == x, x, 0)
    top2 values + indices along time  -> ([32,3,64,2] f32, [32,3,64,2] int32)

Strategy: flatten to 6144 independent rows, shard 768 rows per core across
8 cores (data parallel, no communication).  Per 128-row tile:
  - abs on the scalar engine (in place, in a 0.0-padded buffer)
  - van Herk / Gil-Werman sliding max at half resolution: per-150-block
    prefix/suffix max scans on h[v] = max(x[2v], x[2v+1])
    (tensor_tensor_scan with op=max on DVE)
  - masked scores at PAIR resolution: smax is split per parity into two
    contiguous half-res arrays, each masked with a fused custom-DVE
    select, then combined with one pairwise max ->
    mp[v] = (pair v holds a peak) ? peak value : 0      [128, 4096]
  - top-2 extraction: peaks are >= 151 apart, so every 64-sample block
    (32 pair elements) holds at most one nonzero of mp.  Block "sums"
    (== the peak value, bit exactly) are computed on the otherwise-idle
    TENSOR engine: per 128-wide chunk, PE-transpose -> PSUM, ScalarE
    copy -> SBUF, then a selector matmul accumulates the chunk's four
    64-blocks into a PSUM S^T[128 blocks, 128 rows] tile.  After a
    final transpose, max8 over the 128 block maxes gives the top-8
    peak values, and one find_index8 over the |x| row (the xp tile,
    already resident) gives exact indices (minus the pad offset).
    The max8/find_index8 tail of tile i is emitted after tile i+1's
    scan chain so the in-order DVE stream never waits on the PE chain.

Exact-duplicate f32 values in the input (birthday collisions) need care:
  - two tied peaks in one 64-block make the PE sum a doubled "ghost"
    whose find_index8 lookup misses; run() drops ghosts (idx >= NT).
  - find_index8 duplicate search values return successive occurrences,
    matching jax.lax.top_k's ascending-index order for tied peaks.
"""

import numpy as np

NB, NC, NX, NT = 32, 3, 64, 8192
KERNEL = 301
PAD = KERNEL // 2  # 150
N_CORES = 8
ROWS = NB * NC * NX  # 6144
ROWS_PER_CORE = ROWS // N_CORES  # 768
P_DIM = 128
NTILE = ROWS_PER_CORE // P_DIM  # 6
BMC = 64   # time-domain block size for the block-max top-k (64 < 151)
NBM = NT // BMC  # 128 block maxes per row
BLK = BMC // 2   # 32 pair elements per block
PER = P_DIM // BLK  # 4 blocks per transpose chunk
WOFF = P_DIM - PER

_cached = None
_select_ge = None


def _register_select_ge():
    """Register a fused custom-DVE op  out = (in0 >= in1) ? in0 : 0  at
    runtime (the package's dve_ops.py is read-only; the documented way to
    add an op is appending to its OPS registry)."""
    global _select_ge
    if _select_ge is not None:
        return _select_ge
    import concourse.dve_ops as dve_ops_mod
    from concourse.dve_spec import Spec, Src0, Src1, Zero, select, lower
    from concourse.dve_spec import _has_src1
    from concourse.dve_uop import DveOpSpec

    name = "SELECT_GE_PEAK_ANT"
    for op in dve_ops_mod.OPS:
        if op.name == name:
            _select_ge = op
            return op

    spec = Spec(
        body=select(Src0 >= Src1, Src0, Zero),
        reference=lambda in0, in1, s0, s1, imm2: np.where(
            in0 >= in1, in0, 0.0
        ).astype(np.float32),
    )
    row = dve_ops_mod._CUSTOM_DVE_ROW_BASE + len(dve_ops_mod.OPS)
    assert row < 0x20
    shas = {}
    for ver in ("v3", "v4"):
        s = DveOpSpec(
            name=name, opcode=row, uops=lower(spec, ver=ver), rd1_en=_has_src1(spec)
        )
        shas[ver] = s.sha(ver)
    op = dve_ops_mod.DveOp(name, spec, subdim=False, uops_sha=shas)
    dve_ops_mod.OPS.append(op)
    dve_ops_mod._SUB_OPCODE_FOR_NAME[name] = row
    dve_ops_mod.CUSTOM_DVE_SPECS[name] = spec
    _select_ge = op
    return op


def _build(rows_per_core=ROWS_PER_CORE):
    import concourse.mybir as mybir
    from concourse.bacc import Bacc
    from concourse.tile import TileContext
    from concourse.masks import make_identity

    f32 = mybir.dt.float32
    Alu = mybir.AluOpType
    Act = mybir.ActivationFunctionType
    n_tiles = rows_per_core // P_DIM
    sel_op = _register_select_ge()

    # Bacc (not plain Bass): its finalize() runs generate_event_semaphores,
    # which splits multi-sem waits into EventSemaphore prefixes — TRN2
    # instructions only have a single wait slot.
    nc = Bacc(None, target_bir_lowering=False)
    x_in = nc.dram_tensor("x", [rows_per_core, NT], f32, kind="ExternalInput")
    w_in = nc.dram_tensor("wb", [P_DIM, 2 * P_DIM], f32, kind="ExternalInput")
    out_vals = nc.dram_tensor("out_vals", [rows_per_core, 8], f32, kind="ExternalOutput")
    out_idx = nc.dram_tensor("out_idx", [rows_per_core, 8], mybir.dt.uint32, kind="ExternalOutput")

    # Half-resolution (parity) van Herk: the expensive segmented scans run
    # on h[v] = max(x[2v], x[2v+1]) with window 150 / block 150, then the
    # per-parity sliding max is
    #   smax[2u]   = max(H150[u], xp[2u+300])
    #   smax[2u+1] = max(xp[2u+1], H150[u+1])
    # with H150[v] = max(h[v..v+149]) = max(Sh[v], Ph[v+149]).
    B2 = 150
    MH = NT // 2 + 1        # 4097 H150 values needed
    PHE = B2 - 1 + MH       # 4246, Ph read range
    SHE = (NT // 2 // B2) * B2 + B2 - 1  # 4199, end of Sh's block
    HLEN = PHE + 1          # 4247 h values suffice for both scans
    LP2 = 2 * HLEN          # 8494 xp values (>= PAD + NT + PAD = 8492)
    GLEN = PHE + 1
    HNT = NT // 2

    with TileContext(nc) as tc:
        with (
            tc.tile_pool(name="const", bufs=1) as cpool,
            tc.tile_pool(name="big", bufs=3) as bigpool,
            tc.tile_pool(name="scan", bufs=1) as scanpool,
            tc.tile_pool(name="pair", bufs=2) as pairpool,
            tc.tile_pool(name="small", bufs=2) as smallpool,
            tc.tile_pool(name="stage", bufs=2) as stagepool,
            tc.tile_pool(name="ps", bufs=3, space="PSUM") as pspool,
            tc.tile_pool(name="psS", bufs=1, space="PSUM") as psSpool,
            tc.tile_pool(name="ps2", bufs=2, space="PSUM") as ps2pool,
        ):
            # Segment mask for block-restarting max scans over h: zeros at
            # multiples of 150 (scan state = max(G2[v]*state, h[v]) restarts
            # at every 0 since all data >= 0). G2[1:] reversed provides the
            # restart markers for the reversed (suffix) scan.
            G2 = cpool.tile([P_DIM, GLEN], f32, tag="G2")
            nc.gpsimd.memset(G2[:, :], 1.0)
            nc.gpsimd.memset(G2[:, 0:GLEN:B2], 0.0)
            ident = cpool.tile([P_DIM, P_DIM], f32, tag="ident")
            make_identity(nc, ident[:, :])
            # Wbig[p, k] = 1 iff k == WOFF + p//BLK; the slice
            # Wbig[:, WOFF-PER*c : WOFF-PER*c+128] is the chunk-c selector
            # placing its PER blocks at rows PER*c.. of the accumulated S^T.
            Wbig = cpool.tile([P_DIM, 2 * P_DIM], f32, tag="W")
            nc.sync.dma_start(Wbig[:, :], w_in[:, :])

            prev = None  # deferred tail state: (rows, xint, Ssb)
            for i in range(n_tiles):
                rows = slice(i * P_DIM, (i + 1) * P_DIM)
                xp = bigpool.tile([P_DIM, LP2], f32, tag="xp")
                xint = xp[:, PAD:PAD + NT]
                # Pads + abs all on the scalar engine (|0|=0 keeps pads valid);
                # pads only matter as neutral (<= data) elements.  Tile 0 is
                # fully on the critical path (nothing to overlap with), so
                # chunk its DMA+abs to let compute start sooner.
                h = scanpool.tile([P_DIM, HLEN], f32, tag="h")
                Ph = scanpool.tile([P_DIM, PHE], f32, tag="Ph")
                # Chunked DMA+abs on tile 0 (nothing hides its DMA) so DVE
                # h-build / prefix-scan can start after the first chunk.
                edges = [0, 2048, 4096, 6144, NT] if i == 0 else [0, NT]
                nchunk = len(edges) - 1
                nc.scalar.memzero(xp[:, 0:PAD])
                nc.scalar.memzero(xp[:, PAD + NT:LP2])
                # h-build boundaries per DMA chunk (pair coords); the first
                # chunk also covers the left pad, the last the right pad.
                hb = [0] + [(PAD + edges[c + 1]) // 2 for c in range(nchunk - 1)] + [HLEN]
                pb = [min(b, PHE) for b in hb]
                for c in range(nchunk):
                    sl = slice(PAD + edges[c], PAD + edges[c + 1])
                    nc.sync.dma_start(xp[:, sl], x_in[rows, edges[c]:edges[c + 1]])
                    nc.scalar.activation(xp[:, sl], xp[:, sl], Act.Abs)
                    # h over the freshly-landed pairs, then the prefix scan
                    # over the same range (state chained via `initial`).
                    nc.vector.tensor_tensor(
                        out=h[:, hb[c]:hb[c + 1]],
                        in0=xp[:, 2 * hb[c]:2 * hb[c + 1]:2],
                        in1=xp[:, 2 * hb[c] + 1:2 * hb[c + 1]:2],
                        op=Alu.max,
                    )
                    if pb[c + 1] > pb[c]:
                        nc.vector.tensor_tensor_scan(
                            Ph[:, pb[c]:pb[c + 1]], G2[:, pb[c]:pb[c + 1]],
                            h[:, pb[c]:pb[c + 1]],
                            0.0 if c == 0 else Ph[:, pb[c] - 1:pb[c]],
                            op0=Alu.mult, op1=Alu.max,
                        )

                # Trimmed scan ranges: Ph is only read on [149, 4246) and Sh
                # on [0, 4097).  The suffix scan runs IN PLACE over h (h is
                # dead after the scans; streaming reads each element before
                # its overwrite).
                nc.vector.tensor_tensor_scan(
                    h[:, SHE::-1], G2[:, 1:SHE + 2][:, ::-1], h[:, SHE::-1], 0.0,
                    op0=Alu.mult, op1=Alu.max,
                )

                # H150[v] = max(Sh[v], Ph[v+149]), v in [0, 4097)
                mh = scanpool.tile([P_DIM, MH], f32, tag="mh")
                nc.vector.tensor_tensor(
                    out=mh[:, :], in0=h[:, 0:MH], in1=Ph[:, B2 - 1:B2 - 1 + MH],
                    op=Alu.max,
                )
                # Per-parity sliding max, contiguous half-res outputs.
                # smaxE reuses h's storage, smaxO reuses Ph's (both dead,
                # consumed immediately in order by the selects below).
                nc.vector.tensor_tensor(
                    out=h[:, 0:HNT], in0=mh[:, 0:HNT],
                    in1=xp[:, 2 * PAD:2 * PAD + NT:2], op=Alu.max,
                )
                nc.vector.tensor_tensor(
                    out=Ph[:, 0:HNT], in0=xp[:, 1:NT:2], in1=mh[:, 1:HNT + 1],
                    op=Alu.max,
                )
                # Deferred tail of the previous tile, interleaved into this
                # tile's select chain: the independent max8/find_index8 sit
                # between dependent DVE pairs, hiding pipeline drains, and by
                # now (~35us past the previous mp) its PE chain has drained.
                if prev is not None:
                    pv8 = smallpool.tile([P_DIM, 8], f32, tag="v8")
                    nc.vector.max(out=pv8, in_=prev[2][:, :])
                # Masked scores per parity (fused select), then one pairwise
                # max -> mp.  selE reuses mh's storage, selO overwrites Ph.
                nc.vector._custom_dve(
                    sel_op, out=mh[:, 0:HNT],
                    in0=xp[:, PAD:PAD + NT:2], in1=h[:, 0:HNT],
                )
                nc.vector._custom_dve(
                    sel_op, out=Ph[:, 0:HNT],
                    in0=xp[:, PAD + 1:PAD + NT:2], in1=Ph[:, 0:HNT],
                )
                if prev is not None:
                    pi8 = smallpool.tile([P_DIM, 8], mybir.dt.uint32, tag="i8")
                    nc.vector.max_index(out=pi8, in_max=pv8, in_values=prev[1])
                    nc.sync.dma_start(out_vals[prev[0], :], pv8)
                    nc.sync.dma_start(out_idx[prev[0], :], pi8)
                    prev = None
                mp = pairpool.tile([P_DIM, HNT], f32, tag="mp")
                nc.vector.tensor_tensor(
                    out=mp[:, :], in0=mh[:, 0:HNT], in1=Ph[:, 0:HNT],
                    op=Alu.max,
                )

                if i == n_tiles - 1:
                    # Last tile: nothing left to hide the PE chain behind, so
                    # take the top-8 directly from mp on DVE (4096-elem max8
                    # beats waiting ~20us for the PE pipeline to drain).
                    v8 = smallpool.tile([P_DIM, 8], f32, tag="v8")
                    i8 = smallpool.tile([P_DIM, 8], mybir.dt.uint32, tag="i8")
                    nc.vector.max(out=v8, in_=mp[:, :])
                    nc.vector.max_index(out=i8, in_max=v8, in_values=xint)
                    nc.sync.dma_start(out_vals[rows, :], v8)
                    nc.sync.dma_start(out_idx[rows, :], i8)
                    prev = None
                    continue

                # ---- PE block-max pipeline (no DVE involvement) ----
                ST_ps = ps2pool.tile([P_DIM, P_DIM], f32, tag="ST")
                nch = HNT // P_DIM  # 32
                for c in range(nch):
                    tp = pspool.tile([P_DIM, P_DIM], f32, tag="tp")
                    nc.tensor.transpose(
                        tp[:, :], mp[:, c * P_DIM:(c + 1) * P_DIM], ident[:, :]
                    )
                    ts = stagepool.tile([P_DIM, P_DIM], f32, tag="ts")
                    nc.scalar.activation(ts[:, :], tp[:, :], Act.Copy)
                    nc.tensor.matmul(
                        ST_ps[:, :],
                        Wbig[:, WOFF - PER * c:WOFF - PER * c + P_DIM],
                        ts[:, :],
                        start=(c == 0), stop=(c == nch - 1),
                    )
                STs = stagepool.tile([P_DIM, P_DIM], f32, tag="STs")
                nc.scalar.activation(STs[:, :], ST_ps[:, :], Act.Copy)
                S_ps = psSpool.tile([P_DIM, P_DIM], f32, tag="S")
                nc.tensor.transpose(S_ps[:, :], STs[:, :], ident[:, :])
                Ssb = smallpool.tile([P_DIM, NBM], f32, tag="Ssb")
                nc.scalar.activation(Ssb[:, :], S_ps[:, :], Act.Copy)

                prev = (rows, xint, Ssb)
            if prev is not None:
                _emit_tail(nc, out_vals, out_idx, smallpool, mybir, *prev)
    return nc


def _emit_tail(nc, out_vals, out_idx, smallpool, mybir, rows, xint, Ssb):
    """Top-8 of the 128 block maxes; exact indices by searching the |x| row."""
    v8 = smallpool.tile([P_DIM, 8], mybir.dt.float32, tag="v8")
    i8 = smallpool.tile([P_DIM, 8], mybir.dt.uint32, tag="i8")
    nc.vector.max(out=v8, in_=Ssb[:, :])
    nc.vector.max_index(out=i8, in_max=v8, in_values=xint)
    nc.sync.dma_start(out_vals[rows, :], v8)
    nc.sync.dma_start(out_idx[rows, :], i8)


def _get_module():
    global _cached
    if _cached is None:
        _cached = _build()
        # run_bass_via_pjrt serializes the module as-is; Bacc.finalize()
        # runs register allocation + event-semaphore legalization.
        _cached.finalize()
    return _cached


def run(xcorr: np.ndarray, trace: bool = False, **spmd_kwargs):
    from concourse.bass_utils import run_bass_kernel_spmd

    x = np.ascontiguousarray(np.asarray(xcorr, dtype=np.float32).reshape(ROWS, NT))
    nc = _get_module()
    # Wbig[p, k] = 1 iff k == WOFF + p//BLK (block selector, see _build)
    wb = np.zeros((P_DIM, 2 * P_DIM), np.float32)
    p = np.arange(P_DIM)
    wb[p, WOFF + p // BLK] = 1.0
    in_maps = [
        {"x": x[c * ROWS_PER_CORE:(c + 1) * ROWS_PER_CORE], "wb": wb}
        for c in range(N_CORES)
    ]
    for attempt in range(2):
        res = run_bass_kernel_spmd(
            nc, in_maps, core_ids=list(range(N_CORES)), trace=trace, **spmd_kwargs
        )
        vals8 = np.concatenate([r["out_vals"] for r in res.results], axis=0)
        idx8 = np.concatenate([r["out_idx"] for r in res.results], axis=0)
        # find_index8 indices are relative to the searched AP view (the row
        # interior), so no pad adjustment is needed.  Ghosts (doubled tie
        # values from the PE block sum) miss in find_index8 and return the
        # 0xFFFFFFFF sentinel; drop them, keeping the first two valid
        # candidates per row.
        ghost = idx8 >= np.uint32(NT)
        if ghost.any():
            order = np.argsort(ghost, axis=1, kind="stable")[:, :2]
            vals = np.take_along_axis(vals8, order, 1)
            idx = np.take_along_axis(idx8, order, 1)
        else:
            vals = vals8[:, :2]
            idx = idx8[:, :2]
        # Reference-free sanity check (guards against rare transient device
        # glitches): final indices in range, values strictly positive and
        # non-increasing.  Re-execute once on violation.
        ok = (
            (idx < np.uint32(NT)).all()
            and (vals > 0).all()
            and (vals[:, 0] >= vals[:, 1]).all()
            and np.isfinite(vals).all()
        )
        if ok or attempt == 1:
            break
    topk_score = vals.reshape(NB, NC, NX, 2).astype(np.float32)
    topk_idx = idx.reshape(NB, NC, NX, 2).astype(np.int32)
    return (topk_score, topk_idx), res


def kernel(xcorr: np.ndarray, nlag=None, **_unused):
    out, _ = run(xcorr)
    return out
